# revision 17
# baseline (speedup 1.0000x reference)
"""Trainium2 Bass kernel for nn_BraidCrossing (B=8, T=2048, D=2048, NG=3).

Math notes
----------
reference computes:
    pair  = [x_t, x_{t+1}]                       (B, T-1, 2D)
    h     = gelu(pair @ W1.T + b1)
    logit = h @ W2.T + b2                        (B, T-1, 2*NG)
    scale = mean(softmax(logit, -1), -1)         == 1/(2*NG) EXACTLY (mean of a
                                                 softmax over the same axis)
    P     = x @ Wp.T + bp
    tmp_t = LN(x_t + P_{t-1} * scale)   t>=1 ;  tmp_0 = x_0
    out_t = LN(tmp_t + P_{t+1} * scale) t<=T-2; out_{T-1} = tmp_{T-1}

Because scale is a constant (1/(2*NG); setup has bp=0, gamma=1, beta=0), the
entire W1/W2/gelu branch is dead code.  The device kernel computes
Q = (x @ Wp.T) * scale, then the two chained layernorms.

Key structural tricks (v2):
 * The LN *means* are linear in x, so they are computed on the HOST exactly:
   mu1[t] = (sum_e x[t] + scale * Qsum[t-1]) / D with Qsum[t] = x[t] . rowsum(WpT),
   and mean(LN1_out) == 0 by construction so mu2[t] = scale * Qsum[t+1] / D.
   Only the variances need a device-side quadratic reduction, done with
   ACT activation(Square, accum_out) one pass per LN.  (NOTE:
   vector.tensor_tensor_reduce crashes the exec unit on this runtime —
   NRT_EXEC_UNIT_UNRECOVERABLE — do not use it.)  This removes all
   bn_stats work (88us of DVE in the v1 kernel) and most ACTIVATE work.
 * PSUM evacuation of tile i happens one pipeline step AFTER its matmuls,
   so the PE issues matmuls back-to-back across tiles (PSUM double buffer)
   and the HAM clock gate stays warm.
 * Boundary rows t=0 and t=T-1 are single matvecs -> computed on host.
 * Output is stored bf16 and upcast on the host (halves store traffic).

Precision: GEMM in fp8 e4m3 (DoubleRow, K=256/matmul, fp32 PSUM); LN chain in
bf16 with fp32 statistics.  Measured max rel err ~1.1e-2 (gate 2e-2).

Sharding: data-parallel over batch, one batch per NeuronCore (8 cores).
"""
import numpy as np
import ml_dtypes

import concourse.bass as bass
from concourse import bacc
import concourse.mybir as mybir
import concourse.tile as tile
from concourse.bass_utils import run_bass_kernel_spmd

FP32 = mybir.dt.float32
BF16 = mybir.dt.bfloat16
F8 = mybir.dt.float8e4
AF = mybir.ActivationFunctionType
ALU = mybir.AluOpType
DR = mybir.MatmulPerfMode.DoubleRow

B, T, D = 8, 2048, 2048
P = 128                # partitions
NT = T // P            # 16 t-tiles
NK = D // P            # 16 contraction k-tiles
NKP = NK // 2          # 8 k-pairs (DoubleRow: 256 contraction per matmul)
NE = D // 512          # 4 psum-bank chunks along e
EPS = 1e-5
N_CORES = 8

SX = 16.0              # fp8 pre-scale for x
SW = 1024.0            # fp8 pre-scale for Wp.T
F8NP = ml_dtypes.float8_e4m3
BF16NP = ml_dtypes.bfloat16

_cache = {}


def _build(scale: float):
    # combined scale folded into the PSUM->SBUF copy:
    # PSUM = (x*SX) @ (WpT*SW)  ->  q = PSUM * (scale / (SX*SW))
    qscale = float(scale) / (SX * SW)

    nc = bacc.Bacc("TRN2", target_bir_lowering=False, debug=False)
    xb_d = nc.declare_dram_parameter("xb", [T, D], BF16, isOutput=False)
    # host-tiled transpose: xTt[i, p, k, tt] = x[i*128+tt, k*128+p] (fp8),
    # so lhsT slice [:, 2kp:2kp+2, :] is the DoubleRow stationary operand
    xTt_d = nc.declare_dram_parameter("xTt", [NT, P, NK, P], F8, isOutput=False)
    # w8[kp, p, s, e] = WpT[(2kp+s)*128+p, e] * SW (fp8)
    w8_d = nc.declare_dram_parameter("w8", [NKP, P, 2, D], F8, isOutput=False)
    # host LN stats: columns [mu1 | c1 | mu2 | c2], c = (eps - mu^2) * D
    st_d = nc.declare_dram_parameter("st", [P, 4 * NT], FP32, isOutput=False)
    out_d = nc.declare_dram_parameter("out", [T, D], BF16, isOutput=True)

    xb_ap = xb_d.ap()
    out_ap = out_d.ap()
    xTt_ap = xTt_d.ap()

    with tile.TileContext(nc) as tc:
        with tc.tile_pool(name="wp", bufs=1) as wp_pool, \
             tc.tile_pool(name="xt", bufs=3) as xt_pool, \
             tc.tile_pool(name="q", bufs=4) as q_pool, \
             tc.tile_pool(name="xv", bufs=4) as xv_pool, \
             tc.tile_pool(name="v1", bufs=3) as v1_pool, \
             tc.tile_pool(name="sq", bufs=2) as sq_pool, \
             tc.tile_pool(name="v2", bufs=3) as v2_pool, \
             tc.tile_pool(name="o", bufs=3) as o_pool, \
             tc.tile_pool(name="stat", bufs=4) as stat_pool, \
             tc.tile_pool(name="ps", bufs=2, space="PSUM") as ps_pool:

            st_sb = stat_pool.tile([P, 4 * NT], FP32, tag="st", bufs=1)

            def mu1(i):
                return st_sb[:, i:i + 1]

            def c1(i):
                return st_sb[:, NT + i:NT + i + 1]

            def mu2(i):
                return st_sb[:, 2 * NT + i:2 * NT + i + 1]

            def c2(i):
                return st_sb[:, 3 * NT + i:3 * NT + i + 1]

            # prefetch first lhsT tiles and x rows, then stream the fp8
            # weights kp-ordered across both HWDGE rings: front(0)'s kp=0
            # matmuls start as soon as xt(0) and wp[0] land
            xt_pre = {}
            xt0 = xt_pool.tile([P, NK, P], F8, tag="xt")
            nc.sync.dma_start(out=xt0, in_=xTt_ap[0])
            xt_pre[0] = xt0
            wp = []
            for kp in range(NKP):
                w = wp_pool.tile([P, 2, D], F8, tag=f"wp{kp}", bufs=1)
                eng = nc.scalar if kp % 2 == 0 else nc.sync
                eng.dma_start(out=w, in_=w8_d.ap()[kp])
                wp.append(w)
            xt1 = xt_pool.tile([P, NK, P], F8, tag="xt")
            nc.sync.dma_start(out=xt1, in_=xTt_ap[1])
            xt_pre[1] = xt1
            xv_pre = {}
            xv0 = xv_pool.tile([P, D], BF16, tag="xv")
            nc.sync.dma_start(out=xv0[:P, :], in_=xb_ap[1:1 + P, :])
            xv_pre[0] = xv0
            nc.scalar.dma_start(out=st_sb, in_=st_d.ap())

            qp_of = {}
            q_of = {}
            v1_of = {}
            v2_of = {}
            ns_of = {i: (P if i < NT - 1 else P - 1) for i in range(NT)}
            no_of = {i: (P if i < NT - 1 else P - 2) for i in range(NT)}

            def front(i):
                xt_i = xt_pre.pop(i)
                qp = ps_pool.tile([P, D], FP32, tag="qps", bufs=2)
                for kp in range(NKP):
                    lhsT = xt_i[:, 2 * kp:2 * kp + 2, :]
                    for n in range(NE):
                        nc.tensor.matmul(qp[:, n * 512:(n + 1) * 512],
                                         lhsT,
                                         wp[kp][:, :, n * 512:(n + 1) * 512],
                                         start=(kp == 0), stop=(kp == NKP - 1),
                                         perf_mode=DR)
                qp_of[i] = qp
                if i + 2 < NT:
                    xt_n = xt_pool.tile([P, NK, P], F8, tag="xt")
                    nc.sync.dma_start(out=xt_n, in_=xTt_ap[i + 2])
                    xt_pre[i + 2] = xt_n
                if i + 1 < NT:
                    ns = ns_of[i + 1]
                    xv_n = xv_pool.tile([P, D], BF16, tag="xv")
                    t0 = (i + 1) * P + 1
                    nc.sync.dma_start(out=xv_n[:ns, :], in_=xb_ap[t0:t0 + ns, :])
                    xv_pre[i + 1] = xv_n

            def evac(i, chunks=1):
                # PSUM -> SBUF (scaled, bf16) on DVE; one step after front(i)
                # so no engine dependency ever gates the PE matmul stream.
                # chunks>1 splits along e so chunk c only waits for the
                # matching PSUM n-chunks (n-outer front) -> starts early.
                q_i = q_pool.tile([P, D], BF16, tag="q")
                qp = qp_of.pop(i)
                cw = D // chunks
                for c in range(chunks):
                    nc.vector.tensor_scalar_mul(q_i[:, c * cw:(c + 1) * cw],
                                                qp[:, c * cw:(c + 1) * cw],
                                                qscale)
                q_of[i] = q_i

            rs1_of = {}

            def half1a(i):
                # v1 = x_t + q_{t-1}; S1 = sum(v1^2); rs1 = 1/sqrt(var+eps)
                ns = ns_of[i]
                xv_i = xv_pre.pop(i)
                q_i = q_of[i]
                v1 = v1_pool.tile([P, D], BF16, tag="v1")
                nc.vector.tensor_add(out=v1[:ns], in0=xv_i[:ns], in1=q_i[:ns])
                sq = sq_pool.tile([P, D], BF16, tag="sq")
                s1a = stat_pool.tile([P, 1], FP32, tag="s1a")
                nc.scalar.activation(out=sq[:ns], in_=v1[:ns],
                                     func=AF.Square, accum_out=s1a[:ns])
                u1 = stat_pool.tile([P, 1], FP32, tag="u1")
                nc.vector.tensor_add(out=u1[:ns], in0=s1a[:ns], in1=c1(i)[:ns])
                s1 = stat_pool.tile([P, 1], FP32, tag="s1")
                nc.scalar.activation(out=s1[:ns], in_=u1[:ns], func=AF.Sqrt,
                                     scale=1.0 / D)
                rs1 = stat_pool.tile([P, 1], FP32, tag="rs1")
                nc.vector.reciprocal(out=rs1[:ns], in_=s1[:ns])
                v1_of[i] = v1
                rs1_of[i] = rs1

            def half1b(i):
                # v2 = LN1(v1) + q_{t+1}: apply on DVE, then the 2-partition
                # shifted add via SWDGE accumulate DMA (gpsimd).  HWDGE
                # copies are NOT used here: a small (2-row) HWDGE copy's
                # completion semaphore can arrive ~8us late when the ring
                # goes idle, which serialized the whole drain.
                ns = ns_of[i]
                no2 = no_of[i]
                v1 = v1_of.pop(i)
                rs1 = rs1_of.pop(i)
                v2 = v2_pool.tile([P, D], BF16, tag="v2")
                nc.vector.tensor_scalar(out=v2[:ns], in0=v1[:ns],
                                        scalar1=mu1(i)[:ns], scalar2=rs1[:ns],
                                        op0=ALU.subtract, op1=ALU.mult)
                nc.gpsimd.dma_start(out=v2[0:P - 2, :], in_=q_of[i][2:P, :],
                                    accum_op=ALU.add)
                if i + 1 < NT:
                    nc.gpsimd.dma_start(out=v2[P - 2:P, :],
                                        in_=q_of[i + 1][0:2, :],
                                        accum_op=ALU.add)
                v2_of[i] = v2

            def half2(i):
                no2 = no_of[i]
                v2 = v2_of.pop(i)
                sqf = sq_pool.tile([P, D], BF16, tag="sq2")
                s2a = stat_pool.tile([P, 1], FP32, tag="s2a")
                nc.scalar.activation(out=sqf[:no2], in_=v2[:no2],
                                     func=AF.Square, accum_out=s2a[:no2])
                u2 = stat_pool.tile([P, 1], FP32, tag="u2")
                nc.vector.tensor_add(out=u2[:no2], in0=s2a[:no2],
                                     in1=c2(i)[:no2])
                s2 = stat_pool.tile([P, 1], FP32, tag="s2")
                nc.scalar.activation(out=s2[:no2], in_=u2[:no2], func=AF.Sqrt,
                                     scale=1.0 / D)
                rs2 = stat_pool.tile([P, 1], FP32, tag="rs2")
                nc.vector.reciprocal(out=rs2[:no2], in_=s2[:no2])
                o = o_pool.tile([P, D], BF16, tag="o")
                nc.vector.tensor_scalar(out=o[:no2], in0=v2[:no2],
                                        scalar1=mu2(i)[:no2], scalar2=rs2[:no2],
                                        op0=ALU.subtract, op1=ALU.mult)
                t0 = i * P + 1
                nc.sync.dma_start(out=out_ap[t0:t0 + no2, :], in_=o[:no2])

            # 5-stage software pipeline: front(i) | evac(i)@+1 | half1a(i)@+2
            # | half1b(i)@+3 | half2(i)@+4.  Every op's cross-engine inputs
            # are produced >= 1 step earlier, so the strict per-engine FIFOs
            # never stall the PE matmul stream (HAM stays warm).
            for i in range(NT - 1):
                front(i)
                if i >= 1:
                    evac(i - 1)
                if i >= 2:
                    half1a(i - 2)
                if i >= 3:
                    half1b(i - 3)
                if i >= 4:
                    half2(i - 4)
            # last front + drain: emit all remaining work in estimated
            # ready-time order, pulling later stages forward so the ACT
            # (Squares) / DVE (evac+adds+applies) / gpsimd (shifted
            # accumulates) queues pipeline across tiles without head-blocks
            front(NT - 1)
            evac(NT - 2)
            half1a(NT - 3)
            evac(NT - 1, chunks=2)
            half1a(NT - 2)
            half1b(NT - 4)
            half2(NT - 5)
            half1b(NT - 3)
            half2(NT - 4)
            half1a(NT - 1)
            half1b(NT - 2)
            half2(NT - 3)
            half1b(NT - 1)
            half2(NT - 2)
            half2(NT - 1)

    nc.compile()
    return nc


def _get_program(scale: float):
    key = round(float(scale), 9)
    if key not in _cache:
        _cache[key] = _build(float(scale))
    return _cache[key]


def _identity_ln_params(bp, gamma, beta):
    return (not np.any(bp)) and (not np.any(beta)) and np.all(gamma == 1.0)


def _ln_np(v):
    mu = v.mean(-1, keepdims=True)
    var = ((v - mu) ** 2).mean(-1, keepdims=True)
    return (v - mu) / np.sqrt(var + EPS)


def _reference_numpy(x, W1, b1, W2, b2, Wp, bp, gamma, beta):
    """Exact numpy port of the jax reference (emergency fallback only)."""
    import math

    def ln(v):
        mu = v.mean(-1, keepdims=True)
        var = ((v - mu) ** 2).mean(-1, keepdims=True)
        return (v - mu) / np.sqrt(var + EPS) * gamma + beta

    erf = np.vectorize(math.erf)
    x64 = x.astype(np.float32)
    pair = np.concatenate([x64[:, :-1], x64[:, 1:]], axis=-1)
    h0 = pair @ W1.T + b1
    h = 0.5 * h0 * (1.0 + erf(h0 / np.sqrt(2.0)))
    logits = h @ W2.T + b2
    e = np.exp(logits - logits.max(-1, keepdims=True))
    sm = e / e.sum(-1, keepdims=True)
    scale = sm.mean(-1, keepdims=True)
    Pm = x64 @ Wp.T + bp
    m = Pm[:, 1:] * scale
    mp = Pm[:, :-1] * scale
    tmp = np.concatenate([x64[:, :1], ln(x64[:, 1:] + mp)], axis=1)
    out = np.concatenate([ln(tmp[:, :-1] + m), tmp[:, -1:]], axis=1)
    return out.astype(np.float32)


def _stats_table(x_c, rowsum, scale):
    """Host LN stats for one batch: [P, 4*NT] fp32 (mu1 | c1 | mu2 | c2)."""
    xsum = x_c.sum(-1, dtype=np.float64)
    Qsum = (x_c @ rowsum).astype(np.float64)
    mu1 = np.zeros(T)
    mu1[1:] = (xsum[1:] + scale * Qsum[:-1]) / D
    mu2 = np.zeros(T)
    mu2[:T - 1] = scale * Qsum[1:] / D
    t_idx = np.arange(NT)[None, :] * P + 1 + np.arange(P)[:, None]  # [P, NT]
    ok1 = t_idx <= T - 1
    ok2 = t_idx <= T - 2
    ti = np.minimum(t_idx, T - 1)
    m1 = np.where(ok1, mu1[ti], 0.0)
    m2 = np.where(ok2, mu2[ti], 0.0)
    cc1 = (EPS - m1 ** 2) * D
    cc2 = (EPS - m2 ** 2) * D
    return np.ascontiguousarray(
        np.concatenate([m1, cc1, m2, cc2], axis=1).astype(np.float32))


def run_device(x, wT, scale, trace=False):
    """x: (B,T,D) fp32, wT: (D,D) fp32 (= Wp.T contiguous)."""
    nc = _get_program(scale)
    x8 = np.clip(x * SX, -240.0, 240.0).astype(F8NP)         # (B,T,D) fp8
    w8 = np.ascontiguousarray(
        np.clip(wT * SW, -240.0, 240.0).astype(F8NP)
        .reshape(NKP, 2, P, D).transpose(0, 2, 1, 3))        # (8,128,2,2048)
    rowsum = wT.sum(1).astype(np.float32)
    in_maps = []
    for c in range(N_CORES):
        xb = np.ascontiguousarray(x[c].astype(BF16NP))
        # xTt[i, p, k, tt] = x8[i*128+tt, k*128+p]
        xTb = np.ascontiguousarray(
            x8[c].reshape(NT, P, NK, P).transpose(0, 3, 2, 1))
        st = _stats_table(x[c], rowsum, scale)
        in_maps.append({"xb": xb, "xTt": xTb, "w8": w8, "st": st})
    res = run_bass_kernel_spmd(nc, in_maps, list(range(N_CORES)), trace=trace)
    out = np.empty((B, T, D), np.float32)
    for c in range(N_CORES):
        out[c] = res.results[c]["out"].astype(np.float32)
    # boundary rows t=0 and t=T-1 (single matvecs) on host, exact fp64 LN
    for c in range(N_CORES):
        p1 = x[c, 1] @ wT
        out[c, 0] = _ln_np((x[c, 0] + scale * p1).astype(np.float64))
        pl = x[c, T - 2] @ wT
        out[c, T - 1] = _ln_np((x[c, T - 1] + scale * pl).astype(np.float64))
    return out, res


def kernel(x, W1, b1, W2, b2, Wp, bp, gamma, beta):
    x = np.asarray(x, dtype=np.float32)
    Wp = np.asarray(Wp, dtype=np.float32)
    bp = np.asarray(bp); gamma = np.asarray(gamma); beta = np.asarray(beta)
    b2 = np.asarray(b2)
    if x.shape != (B, T, D) or not _identity_ln_params(bp, gamma, beta):
        return _reference_numpy(np.asarray(x), np.asarray(W1), np.asarray(b1),
                                np.asarray(W2), b2, Wp, bp, gamma, beta)
    scale = 1.0 / float(b2.shape[0])
    wT = np.ascontiguousarray(Wp.T)
    out, _ = run_device(x, wT, scale, trace=False)
    return out


# revision 18
# speedup vs baseline: 1.0783x; 1.0783x over previous
"""Trainium2 Bass kernel for nn_BraidCrossing (B=8, T=2048, D=2048, NG=3).

Math notes
----------
reference computes:
    pair  = [x_t, x_{t+1}]                       (B, T-1, 2D)
    h     = gelu(pair @ W1.T + b1)
    logit = h @ W2.T + b2                        (B, T-1, 2*NG)
    scale = mean(softmax(logit, -1), -1)         == 1/(2*NG) EXACTLY (mean of a
                                                 softmax over the same axis)
    P     = x @ Wp.T + bp
    tmp_t = LN(x_t + P_{t-1} * scale)   t>=1 ;  tmp_0 = x_0
    out_t = LN(tmp_t + P_{t+1} * scale) t<=T-2; out_{T-1} = tmp_{T-1}

Because scale is a constant (1/(2*NG); setup has bp=0, gamma=1, beta=0), the
entire W1/W2/gelu branch is dead code.  The device kernel computes
Q = x @ Wp.T, then the two chained layernorms (scale folded into the adds).

Key structural tricks:
 * LN *means* are linear in x -> computed on the HOST exactly:
   mu1[t] = (sum_e x[t] + scale * Qsum[t-1]) / D, Qsum[t] = x[t] . rowsum(WpT),
   and mean(LN1_out) == 0 by construction so mu2[t] = scale * Qsum[t+1] / D.
   Only the variances need a device-side quadratic reduction, done with
   ACT activation(Square, accum_out), one pass per LN.  (NOTE:
   vector.tensor_tensor_reduce crashes the exec unit on this runtime —
   NRT_EXEC_UNIT_UNRECOVERABLE — do not use it.)
 * UNIFORM 126-row tiles: tile i handles out rows t = i*128+1 .. i*128+126.
   The 2 rows at each tile boundary (t = i*128+127, i*128+128), plus t=0 and
   t=T-1, are single matvecs -> computed on host (32 rows/core, one small
   batched GEMM).  This removes ALL cross-tile dependencies (the shifted-Q
   operand q[t+1] comes from rows 2..127 of the SAME tile's Q) and all tiny
   2-row DMAs (whose completion semaphores measured ~8-10us latency when the
   HWDGE ring idles — that stall serialized the old drain).
 * PSUM evacuation of tile i happens one pipeline step AFTER its matmuls,
   as a raw tensor_copy (scale folded into the downstream STT adds), so no
   engine dependency ever gates the PE matmul stream (HAM stays warm).
 * Output stored bf16, upcast on host (halves store traffic).
 * Matmul loop kp-outer/n-inner: 4 consecutive MMs share one LDWEIGHTS
   content; n-outer ordering makes LDWEIGHTS rate-limiting (216->259ns/MM).

Precision: GEMM in fp8 e4m3 (DoubleRow, K=256/matmul, fp32 PSUM); LN chain in
bf16 with fp32 statistics.  Measured max rel err ~1.3e-2 (gate 2e-2).

Sharding: data-parallel over batch, one batch per NeuronCore (8 cores).
"""
import numpy as np
import ml_dtypes

import concourse.bass as bass
from concourse import bacc
import concourse.mybir as mybir
import concourse.tile as tile
from concourse.bass_utils import run_bass_kernel_spmd

FP32 = mybir.dt.float32
BF16 = mybir.dt.bfloat16
F8 = mybir.dt.float8e4
AF = mybir.ActivationFunctionType
ALU = mybir.AluOpType
DR = mybir.MatmulPerfMode.DoubleRow

B, T, D = 8, 2048, 2048
P = 128                # partitions
NR = P - 2             # 126 device rows per tile
NT = T // P            # 16 t-tiles
NK = D // P            # 16 contraction k-tiles
NKP = NK // 2          # 8 k-pairs (DoubleRow: 256 contraction per matmul)
NE = D // 512          # 4 psum-bank chunks along e
EPS = 1e-5
N_CORES = 8

SX = 16.0              # fp8 pre-scale for x
SW = 1024.0            # fp8 pre-scale for Wp.T
F8NP = ml_dtypes.float8_e4m3
BF16NP = ml_dtypes.bfloat16

_cache = {}


def _build(scale: float):
    # PSUM = (x*SX) @ (WpT*SW); q_raw = copy(PSUM);
    # q = q_raw * qscale applied inside the two STT adds
    qscale = float(scale) / (SX * SW)

    nc = bacc.Bacc("TRN2", target_bir_lowering=False, debug=False)
    xb_d = nc.declare_dram_parameter("xb", [T, D], BF16, isOutput=False)
    # host-tiled transpose: xTt[i, p, k, tt] = x[i*128+tt, k*128+p] (fp8),
    # so lhsT slice [:, 2kp:2kp+2, :] is the DoubleRow stationary operand
    xTt_d = nc.declare_dram_parameter("xTt", [NT, P, NK, P], F8, isOutput=False)
    # w8[kp, p, s, e] = WpT[(2kp+s)*128+p, e] * SW (fp8)
    w8_d = nc.declare_dram_parameter("w8", [NKP, P, 2, D], F8, isOutput=False)
    # host LN stats: columns [mu1 | c1 | mu2 | c2], c = (eps - mu^2) * D
    st_d = nc.declare_dram_parameter("st", [P, 4 * NT], FP32, isOutput=False)
    out_d = nc.declare_dram_parameter("out", [T, D], BF16, isOutput=True)

    xb_ap = xb_d.ap()
    out_ap = out_d.ap()
    xTt_ap = xTt_d.ap()

    with tile.TileContext(nc) as tc:
        with tc.tile_pool(name="wp", bufs=1) as wp_pool, \
             tc.tile_pool(name="xt", bufs=3) as xt_pool, \
             tc.tile_pool(name="q", bufs=3) as q_pool, \
             tc.tile_pool(name="xv", bufs=4) as xv_pool, \
             tc.tile_pool(name="qs", bufs=3) as qs_pool, \
             tc.tile_pool(name="v1", bufs=3) as v1_pool, \
             tc.tile_pool(name="sq", bufs=2) as sq_pool, \
             tc.tile_pool(name="v2", bufs=3) as v2_pool, \
             tc.tile_pool(name="o", bufs=3) as o_pool, \
             tc.tile_pool(name="stat", bufs=4) as stat_pool, \
             tc.tile_pool(name="ps", bufs=2, space="PSUM") as ps_pool:

            st_sb = stat_pool.tile([P, 4 * NT], FP32, tag="st", bufs=1)

            def mu1(i):
                return st_sb[:, i:i + 1]

            def c1(i):
                return st_sb[:, NT + i:NT + i + 1]

            def mu2(i):
                return st_sb[:, 2 * NT + i:2 * NT + i + 1]

            def c2(i):
                return st_sb[:, 3 * NT + i:3 * NT + i + 1]

            # prefetch first lhsT tile and x rows, then stream the fp8
            # weights kp-ordered across both HWDGE rings: front(0)'s kp=0
            # matmuls start as soon as xt(0) and wp[0] land
            xt_pre = {}
            xt0 = xt_pool.tile([P, NK, P], F8, tag="xt")
            nc.sync.dma_start(out=xt0, in_=xTt_ap[0])
            xt_pre[0] = xt0
            wp = []
            for kp in range(NKP):
                w = wp_pool.tile([P, 2, D], F8, tag=f"wp{kp}", bufs=1)
                eng = nc.scalar if kp % 2 == 0 else nc.sync
                eng.dma_start(out=w, in_=w8_d.ap()[kp])
                wp.append(w)
            xt1 = xt_pool.tile([P, NK, P], F8, tag="xt")
            nc.sync.dma_start(out=xt1, in_=xTt_ap[1])
            xt_pre[1] = xt1
            xv_pre = {}
            xv0 = xv_pool.tile([P, D], BF16, tag="xv")
            nc.sync.dma_start(out=xv0[:NR, :], in_=xb_ap[1:1 + NR, :])
            xv_pre[0] = xv0
            nc.scalar.dma_start(out=st_sb, in_=st_d.ap())

            qp_of = {}
            q_of = {}
            qs_of = {}
            v1_of = {}
            v2_of = {}
            rs1_of = {}

            def front(i):
                xt_i = xt_pre.pop(i)
                qp = ps_pool.tile([P, D], FP32, tag="qps", bufs=2)
                for kp in range(NKP):
                    lhsT = xt_i[:, 2 * kp:2 * kp + 2, :]
                    for n in range(NE):
                        nc.tensor.matmul(qp[:, n * 512:(n + 1) * 512],
                                         lhsT,
                                         wp[kp][:, :, n * 512:(n + 1) * 512],
                                         start=(kp == 0), stop=(kp == NKP - 1),
                                         perf_mode=DR)
                qp_of[i] = qp
                if i + 2 < NT:
                    xt_n = xt_pool.tile([P, NK, P], F8, tag="xt")
                    nc.sync.dma_start(out=xt_n, in_=xTt_ap[i + 2])
                    xt_pre[i + 2] = xt_n
                if i + 1 < NT:
                    xv_n = xv_pool.tile([P, D], BF16, tag="xv")
                    t0 = (i + 1) * P + 1
                    nc.sync.dma_start(out=xv_n[:NR, :], in_=xb_ap[t0:t0 + NR, :])
                    xv_pre[i + 1] = xv_n

            def evac(i, chunks=1):
                # PSUM -> SBUF raw copy (bf16) on DVE, one step after front(i)
                q_i = q_pool.tile([P, D], BF16, tag="q")
                qp = qp_of.pop(i)
                cw = D // chunks
                for c in range(chunks):
                    nc.vector.tensor_copy(out=q_i[:, c * cw:(c + 1) * cw],
                                          in_=qp[:, c * cw:(c + 1) * cw])
                q_of[i] = q_i

            def qs_copy(i):
                # shifted-Q operand for the second LN: qs[j] = Q_raw[i*128+2+j]
                # single 126-row HWDGE copy from THIS tile's q only
                qs_i = qs_pool.tile([P, D], BF16, tag="qs")
                nc.sync.dma_start(out=qs_i[0:NR, :], in_=q_of[i][2:P, :])
                qs_of[i] = qs_i

            def half1a(i):
                # v1 = x_t + qscale*Q[t-1]; S1 = sum(v1^2); rs1 = 1/sqrt(var+eps)
                xv_i = xv_pre.pop(i)
                q_i = q_of.pop(i)
                v1 = v1_pool.tile([P, D], BF16, tag="v1")
                nc.vector.scalar_tensor_tensor(out=v1[:NR], in0=q_i[:NR],
                                               scalar=qscale, in1=xv_i[:NR],
                                               op0=ALU.mult, op1=ALU.add)
                sq = sq_pool.tile([P, D], BF16, tag="sq")
                s1a = stat_pool.tile([P, 1], FP32, tag="s1a")
                nc.scalar.activation(out=sq[:NR], in_=v1[:NR],
                                     func=AF.Square, accum_out=s1a[:NR])
                u1 = stat_pool.tile([P, 1], FP32, tag="u1")
                nc.vector.tensor_add(out=u1[:NR], in0=s1a[:NR], in1=c1(i)[:NR])
                s1 = stat_pool.tile([P, 1], FP32, tag="s1")
                nc.scalar.activation(out=s1[:NR], in_=u1[:NR], func=AF.Sqrt,
                                     scale=1.0 / D)
                rs1 = stat_pool.tile([P, 1], FP32, tag="rs1")
                nc.vector.reciprocal(out=rs1[:NR], in_=s1[:NR])
                v1_of[i] = v1
                rs1_of[i] = rs1

            def half1b(i):
                # v2 = LN1(v1) + qscale*Q[t+1]
                v1 = v1_of.pop(i)
                rs1 = rs1_of.pop(i)
                v2 = v2_pool.tile([P, D], BF16, tag="v2")
                nc.vector.tensor_scalar(out=v2[:NR], in0=v1[:NR],
                                        scalar1=mu1(i)[:NR], scalar2=rs1[:NR],
                                        op0=ALU.subtract, op1=ALU.mult)
                nc.vector.scalar_tensor_tensor(out=v2[:NR],
                                               in0=qs_of.pop(i)[:NR],
                                               scalar=qscale, in1=v2[:NR],
                                               op0=ALU.mult, op1=ALU.add)
                v2_of[i] = v2

            def half2(i):
                v2 = v2_of.pop(i)
                sqf = sq_pool.tile([P, D], BF16, tag="sq2")
                s2a = stat_pool.tile([P, 1], FP32, tag="s2a")
                nc.scalar.activation(out=sqf[:NR], in_=v2[:NR],
                                     func=AF.Square, accum_out=s2a[:NR])
                u2 = stat_pool.tile([P, 1], FP32, tag="u2")
                nc.vector.tensor_add(out=u2[:NR], in0=s2a[:NR], in1=c2(i)[:NR])
                s2 = stat_pool.tile([P, 1], FP32, tag="s2")
                nc.scalar.activation(out=s2[:NR], in_=u2[:NR], func=AF.Sqrt,
                                     scale=1.0 / D)
                rs2 = stat_pool.tile([P, 1], FP32, tag="rs2")
                nc.vector.reciprocal(out=rs2[:NR], in_=s2[:NR])
                o = o_pool.tile([P, D], BF16, tag="o")
                nc.vector.tensor_scalar(out=o[:NR], in0=v2[:NR],
                                        scalar1=mu2(i)[:NR], scalar2=rs2[:NR],
                                        op0=ALU.subtract, op1=ALU.mult)
                t0 = i * P + 1
                nc.sync.dma_start(out=out_ap[t0:t0 + NR, :], in_=o[:NR])

            # 5-stage software pipeline: front(i) | evac+qs(i)@+1 |
            # half1a(i)@+2 | half1b(i)@+3 | half2(i)@+4.  Every op's
            # cross-engine inputs are produced >= 1 step earlier, so the
            # strict per-engine FIFOs never stall the PE matmul stream.
            # Tiles are fully independent (no cross-tile data deps).
            for i in range(NT - 1):
                front(i)
                if i >= 1:
                    evac(i - 1)
                    qs_copy(i - 1)
                if i >= 2:
                    half1a(i - 2)
                if i >= 3:
                    half1b(i - 3)
                if i >= 4:
                    half2(i - 4)
            # last front + drain in estimated ready-time order; tile chains
            # are independent so ACT/DVE/sync pipeline across tiles freely
            front(NT - 1)
            evac(NT - 2)
            qs_copy(NT - 2)
            half1a(NT - 3)
            half1b(NT - 4)
            half2(NT - 5)
            evac(NT - 1, chunks=2)
            qs_copy(NT - 1)
            half1a(NT - 2)
            half1b(NT - 3)
            half2(NT - 4)
            half1a(NT - 1)
            half1b(NT - 2)
            half2(NT - 3)
            half1b(NT - 1)
            half2(NT - 2)
            half2(NT - 1)

    nc.compile()
    return nc


def _get_program(scale: float):
    key = round(float(scale), 9)
    if key not in _cache:
        _cache[key] = _build(float(scale))
    return _cache[key]


def _identity_ln_params(bp, gamma, beta):
    return (not np.any(bp)) and (not np.any(beta)) and np.all(gamma == 1.0)


def _ln_np(v):
    mu = v.mean(-1, keepdims=True)
    var = ((v - mu) ** 2).mean(-1, keepdims=True)
    return (v - mu) / np.sqrt(var + EPS)


def _reference_numpy(x, W1, b1, W2, b2, Wp, bp, gamma, beta):
    """Exact numpy port of the jax reference (emergency fallback only)."""
    import math

    def ln(v):
        mu = v.mean(-1, keepdims=True)
        var = ((v - mu) ** 2).mean(-1, keepdims=True)
        return (v - mu) / np.sqrt(var + EPS) * gamma + beta

    erf = np.vectorize(math.erf)
    x64 = x.astype(np.float32)
    pair = np.concatenate([x64[:, :-1], x64[:, 1:]], axis=-1)
    h0 = pair @ W1.T + b1
    h = 0.5 * h0 * (1.0 + erf(h0 / np.sqrt(2.0)))
    logits = h @ W2.T + b2
    e = np.exp(logits - logits.max(-1, keepdims=True))
    sm = e / e.sum(-1, keepdims=True)
    scale = sm.mean(-1, keepdims=True)
    Pm = x64 @ Wp.T + bp
    m = Pm[:, 1:] * scale
    mp = Pm[:, :-1] * scale
    tmp = np.concatenate([x64[:, :1], ln(x64[:, 1:] + mp)], axis=1)
    out = np.concatenate([ln(tmp[:, :-1] + m), tmp[:, -1:]], axis=1)
    return out.astype(np.float32)


# device-skipped rows: t=0, T-1 and the 2 rows at each 126-row tile boundary
HOST_TS = sorted({0, T - 1} |
                 {i * P + 127 for i in range(NT - 1)} |
                 {i * P + 128 for i in range(NT - 1)})
_P_ROWS = sorted({1, T - 2} |
                 {i * P + o for i in range(NT - 1) for o in (126, 127, 128, 129)})
_P_IDX = {r: k for k, r in enumerate(_P_ROWS)}


def _stats_table(x_c, rowsum, scale):
    """Host LN stats for one batch: [P, 4*NT] fp32 (mu1 | c1 | mu2 | c2)."""
    xsum = x_c.sum(-1, dtype=np.float64)
    Qsum = (x_c @ rowsum).astype(np.float64)
    mu1 = np.zeros(T)
    mu1[1:] = (xsum[1:] + scale * Qsum[:-1]) / D
    mu2 = np.zeros(T)
    mu2[:T - 1] = scale * Qsum[1:] / D
    t_idx = np.arange(NT)[None, :] * P + 1 + np.arange(P)[:, None]  # [P, NT]
    ok = np.arange(P)[:, None] < NR
    ti = np.minimum(t_idx, T - 1)
    m1 = np.where(ok, mu1[ti], 0.0)
    m2 = np.where(ok, mu2[ti], 0.0)
    cc1 = (EPS - m1 ** 2) * D
    cc2 = (EPS - m2 ** 2) * D
    return np.ascontiguousarray(
        np.concatenate([m1, cc1, m2, cc2], axis=1).astype(np.float32))


def _host_rows(x, wT, scale, out):
    """Fill the device-skipped rows exactly on host (one batched GEMM)."""
    Pn = x[:, _P_ROWS, :].astype(np.float64) @ wT.astype(np.float64)  # (B,R,D)
    for c in range(N_CORES):
        for t in HOST_TS:
            if t == 0:
                tmp = x[c, 0].astype(np.float64)
            else:
                tmp = _ln_np(x[c, t].astype(np.float64)
                             + scale * Pn[c, _P_IDX[t - 1]])
            if t == T - 1:
                out[c, t] = tmp
            else:
                out[c, t] = _ln_np(tmp + scale * Pn[c, _P_IDX[t + 1]])


def run_device(x, wT, scale, trace=False):
    """x: (B,T,D) fp32, wT: (D,D) fp32 (= Wp.T contiguous)."""
    nc = _get_program(scale)
    x8 = np.clip(x * SX, -240.0, 240.0).astype(F8NP)         # (B,T,D) fp8
    w8 = np.ascontiguousarray(
        np.clip(wT * SW, -240.0, 240.0).astype(F8NP)
        .reshape(NKP, 2, P, D).transpose(0, 2, 1, 3))        # (8,128,2,2048)
    rowsum = wT.sum(1).astype(np.float32)
    in_maps = []
    for c in range(N_CORES):
        xb = np.ascontiguousarray(x[c].astype(BF16NP))
        # xTt[i, p, k, tt] = x8[i*128+tt, k*128+p]
        xTb = np.ascontiguousarray(
            x8[c].reshape(NT, P, NK, P).transpose(0, 3, 2, 1))
        st = _stats_table(x[c], rowsum, scale)
        in_maps.append({"xb": xb, "xTt": xTb, "w8": w8, "st": st})
    res = run_bass_kernel_spmd(nc, in_maps, list(range(N_CORES)), trace=trace)
    out = np.empty((B, T, D), np.float32)
    for c in range(N_CORES):
        out[c] = res.results[c]["out"].astype(np.float32)
    _host_rows(x, wT, scale, out)
    return out, res


def kernel(x, W1, b1, W2, b2, Wp, bp, gamma, beta):
    x = np.asarray(x, dtype=np.float32)
    Wp = np.asarray(Wp, dtype=np.float32)
    bp = np.asarray(bp); gamma = np.asarray(gamma); beta = np.asarray(beta)
    b2 = np.asarray(b2)
    if x.shape != (B, T, D) or not _identity_ln_params(bp, gamma, beta):
        return _reference_numpy(np.asarray(x), np.asarray(W1), np.asarray(b1),
                                np.asarray(W2), b2, Wp, bp, gamma, beta)
    scale = 1.0 / float(b2.shape[0])
    wT = np.ascontiguousarray(Wp.T)
    out, _ = run_device(x, wT, scale, trace=False)
    return out


# revision 19
# speedup vs baseline: 1.2401x; 1.1501x over previous
"""Trainium2 Bass kernel for nn_BraidCrossing (B=8, T=2048, D=2048, NG=3).

Math notes
----------
reference computes:
    pair  = [x_t, x_{t+1}]                       (B, T-1, 2D)
    h     = gelu(pair @ W1.T + b1)
    logit = h @ W2.T + b2                        (B, T-1, 2*NG)
    scale = mean(softmax(logit, -1), -1)         == 1/(2*NG) EXACTLY (mean of a
                                                 softmax over the same axis)
    P     = x @ Wp.T + bp
    tmp_t = LN(x_t + P_{t-1} * scale)   t>=1 ;  tmp_0 = x_0
    out_t = LN(tmp_t + P_{t+1} * scale) t<=T-2; out_{T-1} = tmp_{T-1}

Because scale is a constant (1/(2*NG); setup has bp=0, gamma=1, beta=0), the
entire W1/W2/gelu branch is dead code.  The device kernel computes
Q = x @ Wp.T, then the two chained layernorms (scale folded into the adds).

Key structural tricks:
 * LN *means* are linear in x -> computed on the HOST exactly:
   mu1[t] = (sum_e x[t] + scale * Qsum[t-1]) / D, Qsum[t] = x[t] . rowsum(WpT),
   and mean(LN1_out) == 0 by construction so mu2[t] = scale * Qsum[t+1] / D.
   Only the variances need a device-side quadratic reduction, done with
   ACT activation(Square, accum_out), one pass per LN.  (NOTE:
   vector.tensor_tensor_reduce crashes the exec unit on this runtime —
   NRT_EXEC_UNIT_UNRECOVERABLE — do not use it.)
 * UNIFORM 126-row tiles: tile i handles out rows t = i*128+1 .. i*128+126.
   The 2 rows at each tile boundary (t = i*128+127, i*128+128), plus t=0 and
   t=T-1, are single matvecs -> computed on host (32 rows/core, one small
   batched GEMM).  This removes ALL cross-tile dependencies (the shifted-Q
   operand q[t+1] comes from rows 2..127 of the SAME tile's Q) and all tiny
   2-row DMAs (whose completion semaphores measured ~8-10us latency when the
   HWDGE ring idles — that stall serialized the old drain).
 * PSUM evacuation of tile i happens one pipeline step AFTER its matmuls,
   as a raw tensor_copy (scale folded into the downstream STT adds), so no
   engine dependency ever gates the PE matmul stream (HAM stays warm).
 * Output stored bf16, upcast on host (halves store traffic).
 * Matmul loop kp-outer/n-inner: 4 consecutive MMs share one LDWEIGHTS
   content; n-outer ordering makes LDWEIGHTS rate-limiting (216->259ns/MM).

Precision: GEMM in fp8 e4m3 (DoubleRow, K=256/matmul, fp32 PSUM); LN chain in
bf16 with fp32 statistics.  Measured max rel err ~1.3e-2 (gate 2e-2).

Sharding: data-parallel over batch, one batch per NeuronCore (8 cores).
"""
import numpy as np
import ml_dtypes

import concourse.bass as bass
from concourse import bacc
import concourse.mybir as mybir
import concourse.tile as tile
from concourse.bass_utils import run_bass_kernel_spmd

FP32 = mybir.dt.float32
BF16 = mybir.dt.bfloat16
F8 = mybir.dt.float8e4
AF = mybir.ActivationFunctionType
ALU = mybir.AluOpType
DR = mybir.MatmulPerfMode.DoubleRow

B, T, D = 8, 2048, 2048
P = 128                # partitions
NR = P - 2             # 126 device rows per tile
NT = T // P            # 16 t-tiles
NK = D // P            # 16 contraction k-tiles
NKP = NK // 2          # 8 k-pairs (DoubleRow: 256 contraction per matmul)
NE = D // 512          # 4 psum-bank chunks along e
EPS = 1e-5
N_CORES = 8

SX = 16.0              # fp8 pre-scale for x
SW = 1024.0            # fp8 pre-scale for Wp.T
F8NP = ml_dtypes.float8_e4m3
BF16NP = ml_dtypes.bfloat16

_cache = {}


def _build(scale: float):
    # PSUM = (x*SX) @ (WpT*SW); q_raw = copy(PSUM);
    # q = q_raw * qscale applied inside the two STT adds
    qscale = float(scale) / (SX * SW)

    nc = bacc.Bacc("TRN2", target_bir_lowering=False, debug=False)
    xb_d = nc.declare_dram_parameter("xb", [T, D], BF16, isOutput=False)
    # host-tiled transpose: xTt[i, p, k, tt] = x[i*128+tt, k*128+p] (fp8),
    # so lhsT slice [:, 2kp:2kp+2, :] is the DoubleRow stationary operand
    xTt_d = nc.declare_dram_parameter("xTt", [NT, P, NK, P], F8, isOutput=False)
    # w8[kp, p, s, e] = WpT[(2kp+s)*128+p, e] * SW (fp8)
    w8_d = nc.declare_dram_parameter("w8", [NKP, P, 2, D], F8, isOutput=False)
    # host LN stats: columns [mu1 | c1 | mu2 | c2], c = (eps - mu^2) * D
    st_d = nc.declare_dram_parameter("st", [P, 4 * NT], FP32, isOutput=False)
    out_d = nc.declare_dram_parameter("out", [T, D], BF16, isOutput=True)

    xb_ap = xb_d.ap()
    out_ap = out_d.ap()
    xTt_ap = xTt_d.ap()

    with tile.TileContext(nc) as tc:
        with tc.tile_pool(name="wp", bufs=1) as wp_pool, \
             tc.tile_pool(name="xt", bufs=3) as xt_pool, \
             tc.tile_pool(name="q", bufs=3) as q_pool, \
             tc.tile_pool(name="xv", bufs=4) as xv_pool, \
             tc.tile_pool(name="qs", bufs=3) as qs_pool, \
             tc.tile_pool(name="v1", bufs=3) as v1_pool, \
             tc.tile_pool(name="sq", bufs=2) as sq_pool, \
             tc.tile_pool(name="v2", bufs=3) as v2_pool, \
             tc.tile_pool(name="o", bufs=3) as o_pool, \
             tc.tile_pool(name="stat", bufs=4) as stat_pool, \
             tc.tile_pool(name="ps", bufs=2, space="PSUM") as ps_pool:

            st_sb = stat_pool.tile([P, 4 * NT], FP32, tag="st", bufs=1)

            def mu1(i):
                return st_sb[:, i:i + 1]

            def c1(i):
                return st_sb[:, NT + i:NT + i + 1]

            def mu2(i):
                return st_sb[:, 2 * NT + i:2 * NT + i + 1]

            def c2(i):
                return st_sb[:, 3 * NT + i:3 * NT + i + 1]

            # prefetch first lhsT tile and x rows, then stream the fp8
            # weights kp-ordered across both HWDGE rings: front(0)'s kp=0
            # matmuls start as soon as xt(0) and wp[0] land
            xt_pre = {}
            xt0 = xt_pool.tile([P, NK, P], F8, tag="xt")
            nc.sync.dma_start(out=xt0, in_=xTt_ap[0])
            xt_pre[0] = xt0
            wp = []
            for kp in range(NKP):
                w = wp_pool.tile([P, 2, D], F8, tag=f"wp{kp}", bufs=1)
                eng = nc.scalar if kp % 2 == 0 else nc.sync
                eng.dma_start(out=w, in_=w8_d.ap()[kp])
                wp.append(w)
            xt1 = xt_pool.tile([P, NK, P], F8, tag="xt")
            nc.sync.dma_start(out=xt1, in_=xTt_ap[1])
            xt_pre[1] = xt1
            xv_pre = {}
            xv0 = xv_pool.tile([P, D], BF16, tag="xv")
            nc.sync.dma_start(out=xv0[:NR, :], in_=xb_ap[1:1 + NR, :])
            xv_pre[0] = xv0
            nc.scalar.dma_start(out=st_sb, in_=st_d.ap())

            qp_of = {}
            q_of = {}
            qs_of = {}
            v1_of = {}
            v2_of = {}
            rs1_of = {}

            def front(i):
                xt_i = xt_pre.pop(i)
                qp = ps_pool.tile([P, D], FP32, tag="qps", bufs=2)
                for kp in range(NKP):
                    lhsT = xt_i[:, 2 * kp:2 * kp + 2, :]
                    for n in range(NE):
                        nc.tensor.matmul(qp[:, n * 512:(n + 1) * 512],
                                         lhsT,
                                         wp[kp][:, :, n * 512:(n + 1) * 512],
                                         start=(kp == 0), stop=(kp == NKP - 1),
                                         perf_mode=DR)
                qp_of[i] = qp
                if i + 2 < NT:
                    xt_n = xt_pool.tile([P, NK, P], F8, tag="xt")
                    nc.sync.dma_start(out=xt_n, in_=xTt_ap[i + 2])
                    xt_pre[i + 2] = xt_n
                if i + 1 < NT:
                    xv_n = xv_pool.tile([P, D], BF16, tag="xv")
                    t0 = (i + 1) * P + 1
                    nc.sync.dma_start(out=xv_n[:NR, :], in_=xb_ap[t0:t0 + NR, :])
                    xv_pre[i + 1] = xv_n

            def evac(i, chunks=1):
                # PSUM -> SBUF raw copy (bf16) on DVE, one step after front(i)
                q_i = q_pool.tile([P, D], BF16, tag="q")
                qp = qp_of.pop(i)
                cw = D // chunks
                for c in range(chunks):
                    nc.vector.tensor_copy(out=q_i[:, c * cw:(c + 1) * cw],
                                          in_=qp[:, c * cw:(c + 1) * cw])
                q_of[i] = q_i

            def qs_copy(i):
                # shifted-Q operand for the second LN: qs[j] = Q_raw[i*128+2+j]
                # single 126-row HWDGE copy from THIS tile's q only
                qs_i = qs_pool.tile([P, D], BF16, tag="qs")
                nc.sync.dma_start(out=qs_i[0:NR, :], in_=q_of[i][2:P, :])
                qs_of[i] = qs_i

            def half1a(i):
                # v1 = x_t + qscale*Q[t-1]; S1 = sum(v1^2); rs1 = 1/sqrt(var+eps)
                xv_i = xv_pre.pop(i)
                q_i = q_of.pop(i)
                v1 = v1_pool.tile([P, D], BF16, tag="v1")
                nc.vector.tensor_add(out=v1[:NR], in0=q_i[:NR],
                                     in1=xv_i[:NR])
                sq = sq_pool.tile([P, D], BF16, tag="sq")
                s1a = stat_pool.tile([P, 1], FP32, tag="s1a")
                nc.scalar.activation(out=sq[:NR], in_=v1[:NR],
                                     func=AF.Square, accum_out=s1a[:NR])
                u1 = stat_pool.tile([P, 1], FP32, tag="u1")
                nc.vector.tensor_add(out=u1[:NR], in0=s1a[:NR], in1=c1(i)[:NR])
                s1 = stat_pool.tile([P, 1], FP32, tag="s1")
                nc.scalar.activation(out=s1[:NR], in_=u1[:NR], func=AF.Sqrt,
                                     scale=qscale * qscale / D)
                rs1 = stat_pool.tile([P, 1], FP32, tag="rs1")
                nc.vector.reciprocal(out=rs1[:NR], in_=s1[:NR])
                v1_of[i] = v1
                rs1_of[i] = rs1

            def half1b(i):
                # v2 = LN1(v1) + qscale*Q[t+1]
                v1 = v1_of.pop(i)
                rs1 = rs1_of.pop(i)
                v2 = v2_pool.tile([P, D], BF16, tag="v2")
                nc.vector.tensor_scalar(out=v2[:NR], in0=v1[:NR],
                                        scalar1=mu1(i)[:NR], scalar2=rs1[:NR],
                                        op0=ALU.subtract, op1=ALU.mult)
                nc.vector.tensor_add(out=v2[:NR], in0=qs_of.pop(i)[:NR],
                                     in1=v2[:NR])
                v2_of[i] = v2

            def half2(i):
                v2 = v2_of.pop(i)
                sqf = sq_pool.tile([P, D], BF16, tag="sq2")
                s2a = stat_pool.tile([P, 1], FP32, tag="s2a")
                nc.scalar.activation(out=sqf[:NR], in_=v2[:NR],
                                     func=AF.Square, accum_out=s2a[:NR])
                u2 = stat_pool.tile([P, 1], FP32, tag="u2")
                nc.vector.tensor_add(out=u2[:NR], in0=s2a[:NR], in1=c2(i)[:NR])
                s2 = stat_pool.tile([P, 1], FP32, tag="s2")
                nc.scalar.activation(out=s2[:NR], in_=u2[:NR], func=AF.Sqrt,
                                     scale=qscale * qscale / D)
                rs2 = stat_pool.tile([P, 1], FP32, tag="rs2")
                nc.vector.reciprocal(out=rs2[:NR], in_=s2[:NR])
                # rs2' = qscale/sigma2: converts primed v2 back to unit scale
                rs2p = stat_pool.tile([P, 1], FP32, tag="rs2p")
                nc.vector.tensor_scalar_mul(rs2p[:NR], rs2[:NR], qscale)
                o = o_pool.tile([P, D], BF16, tag="o")
                nc.vector.tensor_scalar(out=o[:NR], in0=v2[:NR],
                                        scalar1=mu2(i)[:NR], scalar2=rs2p[:NR],
                                        op0=ALU.subtract, op1=ALU.mult)
                t0 = i * P + 1
                nc.sync.dma_start(out=out_ap[t0:t0 + NR, :], in_=o[:NR])

            # 5-stage software pipeline: front(i) | evac+qs(i)@+1 |
            # half1a(i)@+2 | half1b(i)@+3 | half2(i)@+4.  Every op's
            # cross-engine inputs are produced >= 1 step earlier, so the
            # strict per-engine FIFOs never stall the PE matmul stream.
            # Tiles are fully independent (no cross-tile data deps).
            for i in range(NT - 1):
                front(i)
                if i >= 1:
                    evac(i - 1)
                    qs_copy(i - 1)
                if i >= 2:
                    half1a(i - 2)
                if i >= 3:
                    half1b(i - 3)
                if i >= 4:
                    half2(i - 4)
            # last front + drain in estimated ready-time order; tile chains
            # are independent so ACT/DVE/sync pipeline across tiles freely
            front(NT - 1)
            evac(NT - 2)
            qs_copy(NT - 2)
            half1a(NT - 3)
            half1b(NT - 4)
            half2(NT - 5)
            evac(NT - 1, chunks=2)
            qs_copy(NT - 1)
            half1a(NT - 2)
            half1b(NT - 3)
            half2(NT - 4)
            half1a(NT - 1)
            half1b(NT - 2)
            half2(NT - 3)
            half1b(NT - 1)
            half2(NT - 2)
            half2(NT - 1)

    nc.compile()
    return nc


def _get_program(scale: float):
    key = round(float(scale), 9)
    if key not in _cache:
        _cache[key] = _build(float(scale))
    return _cache[key]


def _identity_ln_params(bp, gamma, beta):
    return (not np.any(bp)) and (not np.any(beta)) and np.all(gamma == 1.0)


def _ln_np(v):
    mu = v.mean(-1, keepdims=True)
    var = ((v - mu) ** 2).mean(-1, keepdims=True)
    return (v - mu) / np.sqrt(var + EPS)


def _reference_numpy(x, W1, b1, W2, b2, Wp, bp, gamma, beta):
    """Exact numpy port of the jax reference (emergency fallback only)."""
    import math

    def ln(v):
        mu = v.mean(-1, keepdims=True)
        var = ((v - mu) ** 2).mean(-1, keepdims=True)
        return (v - mu) / np.sqrt(var + EPS) * gamma + beta

    erf = np.vectorize(math.erf)
    x64 = x.astype(np.float32)
    pair = np.concatenate([x64[:, :-1], x64[:, 1:]], axis=-1)
    h0 = pair @ W1.T + b1
    h = 0.5 * h0 * (1.0 + erf(h0 / np.sqrt(2.0)))
    logits = h @ W2.T + b2
    e = np.exp(logits - logits.max(-1, keepdims=True))
    sm = e / e.sum(-1, keepdims=True)
    scale = sm.mean(-1, keepdims=True)
    Pm = x64 @ Wp.T + bp
    m = Pm[:, 1:] * scale
    mp = Pm[:, :-1] * scale
    tmp = np.concatenate([x64[:, :1], ln(x64[:, 1:] + mp)], axis=1)
    out = np.concatenate([ln(tmp[:, :-1] + m), tmp[:, -1:]], axis=1)
    return out.astype(np.float32)


# device-skipped rows: t=0, T-1 and the 2 rows at each 126-row tile boundary
HOST_TS = sorted({0, T - 1} |
                 {i * P + 127 for i in range(NT - 1)} |
                 {i * P + 128 for i in range(NT - 1)})
_P_ROWS = sorted({1, T - 2} |
                 {i * P + o for i in range(NT - 1) for o in (126, 127, 128, 129)})
_P_IDX = {r: k for k, r in enumerate(_P_ROWS)}


def _stats_table(x_c, rowsum, scale, qscale):
    """Host LN stats in PRIMED units (v' = v/qscale): [P, 4*NT] fp32."""
    xsum = x_c.sum(-1, dtype=np.float64)
    Qsum = (x_c @ rowsum).astype(np.float64)
    mu1 = np.zeros(T)
    mu1[1:] = (xsum[1:] + scale * Qsum[:-1]) / D
    mu2 = np.zeros(T)
    mu2[:T - 1] = scale * Qsum[1:] / D
    t_idx = np.arange(NT)[None, :] * P + 1 + np.arange(P)[:, None]  # [P, NT]
    ok = np.arange(P)[:, None] < NR
    ti = np.minimum(t_idx, T - 1)
    m1 = np.where(ok, mu1[ti], 0.0)
    m2 = np.where(ok, mu2[ti], 0.0)
    cc1 = (EPS - m1 ** 2) * D
    cc2 = (EPS - m2 ** 2) * D
    q2 = qscale * qscale
    return np.ascontiguousarray(
        np.concatenate([m1 / qscale, cc1 / q2, m2 / qscale, cc2 / q2],
                       axis=1).astype(np.float32))


def _host_rows(x, wT, scale, out):
    """Fill the device-skipped rows exactly on host (one batched GEMM)."""
    Pn = x[:, _P_ROWS, :].astype(np.float64) @ wT.astype(np.float64)  # (B,R,D)
    for c in range(N_CORES):
        for t in HOST_TS:
            if t == 0:
                tmp = x[c, 0].astype(np.float64)
            else:
                tmp = _ln_np(x[c, t].astype(np.float64)
                             + scale * Pn[c, _P_IDX[t - 1]])
            if t == T - 1:
                out[c, t] = tmp
            else:
                out[c, t] = _ln_np(tmp + scale * Pn[c, _P_IDX[t + 1]])


def run_device(x, wT, scale, trace=False):
    """x: (B,T,D) fp32, wT: (D,D) fp32 (= Wp.T contiguous)."""
    nc = _get_program(scale)
    x8 = np.clip(x * SX, -240.0, 240.0).astype(F8NP)         # (B,T,D) fp8
    w8 = np.ascontiguousarray(
        np.clip(wT * SW, -240.0, 240.0).astype(F8NP)
        .reshape(NKP, 2, P, D).transpose(0, 2, 1, 3))        # (8,128,2,2048)
    rowsum = wT.sum(1).astype(np.float32)
    qscale = float(scale) / (SX * SW)
    in_maps = []
    for c in range(N_CORES):
        # x in primed units (x/qscale) so the device adds need no scaling
        xb = np.ascontiguousarray((x[c] * (1.0 / qscale)).astype(BF16NP))
        # xTt[i, p, k, tt] = x8[i*128+tt, k*128+p]
        xTb = np.ascontiguousarray(
            x8[c].reshape(NT, P, NK, P).transpose(0, 3, 2, 1))
        st = _stats_table(x[c], rowsum, scale, qscale)
        in_maps.append({"xb": xb, "xTt": xTb, "w8": w8, "st": st})
    res = run_bass_kernel_spmd(nc, in_maps, list(range(N_CORES)), trace=trace)
    out = np.empty((B, T, D), np.float32)
    for c in range(N_CORES):
        out[c] = res.results[c]["out"].astype(np.float32)
    _host_rows(x, wT, scale, out)
    return out, res


def kernel(x, W1, b1, W2, b2, Wp, bp, gamma, beta):
    x = np.asarray(x, dtype=np.float32)
    Wp = np.asarray(Wp, dtype=np.float32)
    bp = np.asarray(bp); gamma = np.asarray(gamma); beta = np.asarray(beta)
    b2 = np.asarray(b2)
    if x.shape != (B, T, D) or not _identity_ln_params(bp, gamma, beta):
        return _reference_numpy(np.asarray(x), np.asarray(W1), np.asarray(b1),
                                np.asarray(W2), b2, Wp, bp, gamma, beta)
    scale = 1.0 / float(b2.shape[0])
    wT = np.ascontiguousarray(Wp.T)
    out, _ = run_device(x, wT, scale, trace=False)
    return out
